# revision 50
# baseline (speedup 1.0000x reference)
"""Multi-head attention (B=2, S=2048, D=1024, H=16) on 8 Trainium2 NeuronCores.

Sharding: 2-way data parallel over batch x 4-way tensor parallel over heads.
Core c handles batch c//4 and heads [4*(c%4), 4*(c%4)+4).  Each core computes
its 4 heads' attention and a partial output projection; the host sums the 4
partials per batch element (the bias bo is only added by the g==0 cores).

Kernel structure:
- Scores per head-pair are two PE-row-tiled matmuls (contraction 64) that
  share the array; attention-times-V uses the fused ones-column (65-wide
  output: 64 v-dims + running sum-exp) accumulated in PSUM per 4-key-block
  round, staged to SBUF fp32 accumulators.
- All data paths (x, weights, qh/kh/vh/pt, at/wo) are bf16: halves SBUF +
  DMA traffic and enables fast weight loads; accumulation stays fp32.
  Measured absmax rel err ~8e-3 vs the 2e-2 gate.
- x tiles prefetch quarters ahead on a deep pool; Q projections front-load
  under the first attention block's exp backlog; K/V quarter jq+1 projects
  in half-chunks spread across round jq's attention blocks.
- The output bias folds into the projection matmul via a ones-row; the last
  quarter's PSUM drains run on ScalarE, which idles after the final exp.
"""

from contextlib import ExitStack

import numpy as np

import concourse.mybir as mybir
import concourse.tile as tile
from concourse import bacc
from concourse import bass_utils
from concourse._compat import with_exitstack

F32 = mybir.dt.float32
F32R = mybir.dt.float32r
BF16 = mybir.dt.bfloat16

DT_MODE = "f32r"

D_MODEL = 1024
N_HEAD = 16
DK = 64
B = 2
S = 2048
N_CORES = 8
HPC = 4          # heads per core
DPC = HPC * DK   # 256 output dims per core
KC = D_MODEL // 128   # 8 contraction chunks of 128
SQ = 512         # sequence quarter
NSQ = S // SQ    # 4
NJB = S // 128   # 16 key blocks

import ml_dtypes

if DT_MODE == "bf16":
    SB_DT = BF16
    IO_NP = ml_dtypes.bfloat16
else:
    # float32r = fp32 storage, single-pass (rounded) PE matmul at bf16 speed.
    SB_DT = F32R
    IO_NP = np.float32

# bf16 everywhere on the attention and projection data paths (x, weights,
# qh/kh/vh/pt, at/wo): halves SBUF + DMA traffic and enables fast weight
# loads; accumulation stays fp32 in PSUM / SBUF accumulators.
QK_DT = BF16
AV_DT = BF16
OD_DT = BF16
X_DT = BF16
X_NP = ml_dtypes.bfloat16


@with_exitstack
def build_mha(ctx: ExitStack, tc, ins, out_ap, loop_n=None):
    """Emit the per-core kernel.  loop_n > 1 wraps the compute body in a
    hardware For_i loop (timing); loop_n < 0 emits -loop_n python-unrolled
    copies (simulator-only steady-state estimate)."""
    nc = tc.nc
    P = 128
    Exp = mybir.ActivationFunctionType.Exp
    Add = mybir.AluOpType.add

    xq = ins["xq_t"].rearrange("(kc p) s -> p kc s", p=P)
    xk = ins["xk_t"].rearrange("(kc p) s -> p kc s", p=P)
    xv = ins["xv_t"].rearrange("(kc p) s -> p kc s", p=P)
    out = out_ap.rearrange("(sb p) n -> p sb n", p=P)

    ec = ctx.enter_context
    cpool = ec(tc.tile_pool(name="consts", bufs=1))
    xpool = ec(tc.tile_pool(name="xs", bufs=4))
    qkpool = ec(tc.tile_pool(name="qk", bufs=1))
    vpool = ec(tc.tile_pool(name="vh", bufs=1))
    ptpool = ec(tc.tile_pool(name="pt", bufs=6))
    apool = ec(tc.tile_pool(name="attn", bufs=1))
    opool = ec(tc.tile_pool(name="outs", bufs=2))
    npool = ec(tc.tile_pool(name="nrm", bufs=3))
    accpool = ec(tc.tile_pool(name="acc", bufs=1))
    pp_ps = ec(tc.tile_pool(name="proj_ps", bufs=2, space="PSUM"))
    sc_ps = ec(tc.tile_pool(name="score_ps", bufs=2, space="PSUM"))
    at_ps = ec(tc.tile_pool(name="att_ps", bufs=2, space="PSUM"))

    # --- constants ---
    wq_sb = cpool.tile([P, KC, DPC], X_DT, tag="wq")
    wk_sb = cpool.tile([P, KC, DPC], X_DT, tag="wk")
    wv_sb = cpool.tile([P, KC, DPC], X_DT, tag="wv")
    wo_sb = cpool.tile([P, 2, D_MODEL], OD_DT, tag="wo")
    # per-chunk weight loads on the scalar-engine DMA queue: the first K
    # projection matmuls only wait for their own chunk, and the x-tile
    # stream (sync queue) runs in parallel.  wo is only needed at the end.
    wq_ap = ins["wq_t"].rearrange("(kc p) m -> p kc m", p=P)
    wk_ap = ins["wk_t"].rearrange("(kc p) m -> p kc m", p=P)
    wv_ap = ins["wv_t"].rearrange("(kc p) m -> p kc m", p=P)
    for kc in range(KC):
        nc.scalar.dma_start(wk_sb[:, kc, :], wk_ap[:, kc, :])
        nc.scalar.dma_start(wq_sb[:, kc, :], wq_ap[:, kc, :])
    for kc in range(KC):
        nc.scalar.dma_start(wv_sb[:, kc, :], wv_ap[:, kc, :])
    nc.gpsimd.dma_start(wo_sb[:], ins["wo_t"].rearrange("(c p) n -> p c n", p=P))
    bq_sb = cpool.tile([P, 2], F32, tag="bq")
    bk_sb = cpool.tile([P, 2], F32, tag="bk")
    bv_sb = cpool.tile([P, DPC], F32, tag="bv")
    bo_sb = cpool.tile([P, D_MODEL], F32, tag="bo")
    nc.gpsimd.dma_start(bq_sb[:], ins["bq_p"][:])
    nc.gpsimd.dma_start(bk_sb[:], ins["bk_p"][:])
    nc.gpsimd.dma_start(bv_sb[:], ins["bv_b"][:])
    nc.gpsimd.dma_start(bo_sb[:], ins["bo_b"][:])

    # --- persistent activations ---
    qh_sb = qkpool.tile([P, 2, S], QK_DT, tag="qh")   # [dk%128, head_pair, s]
    kh_sb = qkpool.tile([P, 2, S], QK_DT, tag="kh")
    vh_sb = vpool.tile([P, NJB, HPC, DK + 1], AV_DT, tag="vh")  # + ones col
    at_sb = apool.tile([P, 2, S], OD_DT, tag="at")    # attn out, transposed

    # walrus can't memset float32r; memset f32 then broadcast-copy
    ones1 = cpool.tile([P, 1], F32, tag="ones1")
    nc.vector.memset(ones1[:], 1.0)
    nc.vector.tensor_copy(
        vh_sb[:, :, :, DK : DK + 1],
        ones1[:, None, None, :].to_broadcast((P, NJB, HPC, 1)),
    )
    # ones row + bo row for folding the output bias into the projection
    # matmul (so the PSUM drain is a pure copy and can run on ScalarE)
    ones_r = cpool.tile([1, P], OD_DT, tag="ones_r")
    nc.vector.tensor_copy(ones_r[:], ones1[0:1, :].to_broadcast((1, P)))
    bo_r = cpool.tile([1, D_MODEL], OD_DT, tag="bo_r")
    nc.vector.tensor_copy(bo_r[:], bo_sb[0:1, :])

    def _x_prefetch(x_ap, sq):
        # one 3D DMA per quarter: 8x fewer sync-queue ops; the quarter-ahead
        # prefetch distance hides the atomic completion
        xt = xpool.tile([P, KC, SQ], X_DT, tag="xt")
        nc.sync.dma_start(xt[:], x_ap[:, :, sq * SQ : (sq + 1) * SQ])
        return xt

    def _qk_quarter(xts, w_sb, b_sb, dst, sq, halves=(0, 1)):
        """Project output-halves (head pairs) of one 512-token quarter."""
        pss = {h: pp_ps.tile([P, 512], F32, tag="pp", name=f"ps{h}")
               for h in halves}
        for kc in range(KC):
            for half in halves:
                nc.tensor.matmul(
                    pss[half][:],
                    w_sb[:, kc, 128 * half : 128 * (half + 1)],
                    xts[:, kc, :],
                    start=(kc == 0), stop=(kc == KC - 1),
                )
        for half in halves:
            nc.vector.tensor_scalar_add(
                dst[:, half, sq * SQ : (sq + 1) * SQ], pss[half][:],
                b_sb[:, half : half + 1],
            )

    def _v_quarter(xts, sq, halves=(0, 1)):
        # natural layout [s, dv]; two row-blocks per PSUM tile run their
        # accumulation groups over pre-loaded k-chunks
        pss = {h: pp_ps.tile([P, 512], F32, tag="pp", name=f"ps{h}")
               for h in halves}
        sbis = [s for h in halves for s in (2 * h, 2 * h + 1)]
        # sbi-outer: the two 256-col accumulation groups sharing a PSUM bank
        # must run sequentially (a second in-flight `start` in the same bank
        # zero-region corrupts the first group's partials)
        for sbi in sbis:
            for kc in range(KC):
                nc.tensor.matmul(
                    pss[sbi // 2][:, (sbi % 2) * 256 : (sbi % 2) * 256 + 256],
                    xts[:, kc, sbi * 128 : (sbi + 1) * 128],
                    wv_sb[:, kc, :],
                    start=(kc == 0), stop=(kc == KC - 1),
                )
        for sbi in sbis:
            jb = sq * 4 + sbi
            nc.vector.tensor_tensor(
                vh_sb[:, jb, :, 0:DK],
                pss[sbi // 2][:, (sbi % 2) * 256 : (sbi % 2) * 256 + 256]
                .rearrange("p (h d) -> p h d", h=HPC),
                bv_sb[:].rearrange("p (h d) -> p h d", h=HPC),
                Add,
            )

    # attention partial accumulators, one per (query quarter, head);
    # row 64 carries the running sum(exp) for the softmax denominator
    acc_sb = [
        [accpool.tile([65, 512], F32, tag=f"acc{i5}_{h}", name=f"acc{i5}_{h}")
         for h in range(HPC)]
        for i5 in range(NSQ)
    ]

    def _attn_block(i5, t, jq):
        """4 key-blocks of attention for head pair t, query quarter i5."""
        i_sl = slice(i5 * SQ, (i5 + 1) * SQ)
        att_e = at_ps.tile([P, 512], F32, tag="att", name="att_e")
        att_o = at_ps.tile([P, 512], F32, tag="att", name="att_o")
        pts = []
        jbs = range(jq * 4, jq * 4 + 4)
        for n, jb in enumerate(jbs):
            sc = sc_ps.tile([P, 1024], F32, tag="sc")
            j_sl = slice(jb * 128, (jb + 1) * 128)
            nc.tensor.matmul(
                sc[:, 0:512], kh_sb[0:64, t, j_sl], qh_sb[0:64, t, i_sl],
                start=True, stop=True,
            )
            nc.tensor.matmul(
                sc[:, 512:1024], kh_sb[64:128, t, j_sl],
                qh_sb[64:128, t, i_sl], start=True, stop=True,
            )
            pt = ptpool.tile([P, 1024], AV_DT, tag="pt")
            nc.scalar.activation(pt[:], sc[:], Exp, scale=1.0 / np.sqrt(DK))
            pts.append(pt)
            if n > 0:
                ptp = pts[n - 1]
                nc.tensor.matmul(
                    att_e[0:65, :], vh_sb[:, jb - 1, 2 * t, :],
                    ptp[:, 0:512], start=(n - 1 == 0), stop=False,
                )
                nc.tensor.matmul(
                    att_o[0:65, :], vh_sb[:, jb - 1, 2 * t + 1, :],
                    ptp[:, 512:1024], start=(n - 1 == 0), stop=False,
                )
        jb_last = jq * 4 + 3
        nc.tensor.matmul(
            att_e[0:65, :], vh_sb[:, jb_last, 2 * t, :],
            pts[-1][:, 0:512], start=False, stop=True,
        )
        nc.tensor.matmul(
            att_o[0:65, :], vh_sb[:, jb_last, 2 * t + 1, :],
            pts[-1][:, 512:1024], start=False, stop=True,
        )
        for h, aps in ((2 * t, att_e), (2 * t + 1, att_o)):
            acc = acc_sb[i5][h]
            if jq == 0:
                nc.vector.tensor_copy(acc[:], aps[0:65, :])
            else:
                nc.vector.tensor_tensor(acc[:], acc[:], aps[0:65, :], Add)

    def _normalize(i5):
        i_sl = slice(i5 * SQ, (i5 + 1) * SQ)
        for h in range(HPC):
            acc = acc_sb[i5][h]
            t = h // 2
            rc = npool.tile([1, 512], F32, tag="rc")
            nc.vector.reciprocal(rc[:], acc[64:65, :])
            bc = npool.tile([64, 512], F32, tag="bc")
            nc.gpsimd.partition_broadcast(bc[:], rc[:])
            if h % 2 == 0:
                nc.vector.tensor_mul(at_sb[0:64, t, i_sl], acc[0:64, :], bc[:])
            else:
                tm = npool.tile([64, 512], OD_DT, tag="tm")
                nc.vector.tensor_mul(tm[:], acc[0:64, :], bc[:])
                nc.sync.dma_start(at_sb[64:128, t, i_sl], tm[:])

    def _final(i5):
        # half-granular so consecutive halves pipeline through the 2 pp
        # bufs; bo is folded into the matmul via a ones-row so the PSUM
        # drain is a pure copy.  The last quarter's copies run on ScalarE,
        # which is idle after the final exp, unloading the vector engine
        # during the tail.
        on_act = i5 == NSQ - 1
        for sbi in range(4):
            sb = i5 * 4 + sbi
            s_sl = slice(sb * 128, (sb + 1) * 128)
            for half in range(2):
                h_sl = slice(512 * half, 512 * half + 512)
                # last quarter: alternate po between the pp pool and the
                # (by now idle) att pool for a 4-deep drain pipeline
                if on_act and sbi % 2:
                    po = at_ps.tile([P, 512], F32, tag="att", name="po")
                else:
                    po = pp_ps.tile([P, 512], F32, tag="pp", name="po")
                nc.tensor.matmul(
                    po[:], ones_r[:], bo_r[:, h_sl], start=True, stop=False)
                for c in range(2):
                    nc.tensor.matmul(
                        po[:], at_sb[:, c, s_sl], wo_sb[:, c, h_sl],
                        start=False, stop=(c == 1),
                    )
                ot = opool.tile([P, 512], F32, tag="ot")
                if on_act:
                    nc.scalar.copy(ot[:], po[:])
                else:
                    nc.vector.tensor_copy(ot[:], po[:])
                nc.sync.dma_start(out[:, sb, h_sl], ot[:])

    def _attn_round(i5, jq):
        for t in range(2):
            _attn_block(i5, t, jq)

    def _compute():
        # Startup: K0 and Q0 first so the exp stream starts as early as
        # possible; V0 lands while the first scores run.
        kx = _x_prefetch(xk, 0)
        qx0 = _x_prefetch(xq, 0)
        _qk_quarter(kx, wk_sb, bk_sb, kh_sb, 0)
        _qk_quarter(qx0, wq_sb, bq_sb, qh_sb, 0)
        vx = _x_prefetch(xv, 0)
        _v_quarter(vx, 0)
        # Round 0: front-load Q1-3 projections under the first attention
        # block's exp backlog, then spread K1/V1 halves across the round.
        _attn_round(0, 0)
        for i5 in range(1, NSQ):
            qx = _x_prefetch(xq, i5)
            _qk_quarter(qx, wq_sb, bq_sb, qh_sb, i5)
        kx = _x_prefetch(xk, 1)
        vx = _x_prefetch(xv, 1)
        _attn_round(1, 0)
        _qk_quarter(kx, wk_sb, bk_sb, kh_sb, 1, halves=(0,))
        _attn_round(2, 0)
        _qk_quarter(kx, wk_sb, bk_sb, kh_sb, 1, halves=(1,))
        _v_quarter(vx, 1, halves=(0,))
        _attn_round(3, 0)
        _v_quarter(vx, 1, halves=(1,))
        # Rounds 1..3
        for jq in range(1, NSQ):
            last = jq == NSQ - 1
            if not last:
                kx = _x_prefetch(xk, jq + 1)
                vx = _x_prefetch(xv, jq + 1)
            for i5 in range(NSQ):
                _attn_round(i5, jq)
                if not last:
                    if i5 == 0:
                        _qk_quarter(kx, wk_sb, bk_sb, kh_sb, jq + 1,
                                    halves=(0,))
                    elif i5 == 1:
                        _qk_quarter(kx, wk_sb, bk_sb, kh_sb, jq + 1,
                                    halves=(1,))
                    elif i5 == 2:
                        _v_quarter(vx, jq + 1, halves=(0,))
                    else:
                        _v_quarter(vx, jq + 1, halves=(1,))
                else:
                    _normalize(i5)
                    _final(i5)

    if loop_n is not None and loop_n > 1:
        with tc.For_i(0, loop_n, 1):
            _compute()
    elif loop_n is not None and loop_n < 0:
        # python-unrolled copies (simulator-only steady-state estimate)
        for _ in range(-loop_n):
            _compute()
    else:
        _compute()


def shard_inputs(q, k, v, Wq, bq, Wk, bk, Wv, bv, Wo, bo):
    """Build the 8 per-core input maps from the full inputs."""
    def prep(a):
        return np.ascontiguousarray(np.asarray(a, np.float32)).astype(
            ml_dtypes.bfloat16)

    in_maps = []
    for c in range(N_CORES):
        b, g = divmod(c, 4)
        hs = slice(g * DPC, (g + 1) * DPC)
        bo_b = (
            np.broadcast_to(np.asarray(bo, np.float32), (128, D_MODEL))
            if g == 0
            else np.zeros((128, D_MODEL), np.float32)
        )
        in_maps.append({
            "xq_t": prep(np.asarray(q)[b].T),
            "xk_t": prep(np.asarray(k)[b].T),
            "xv_t": prep(np.asarray(v)[b].T),
            "wq_t": prep(np.asarray(Wq)[hs, :].T),
            "wk_t": prep(np.asarray(Wk)[hs, :].T),
            "wv_t": prep(np.asarray(Wv)[hs, :].T),
            "wo_t": np.ascontiguousarray(
                np.asarray(Wo, np.float32)[:, hs].T).astype(ml_dtypes.bfloat16),
            "bq_p": np.ascontiguousarray(
                np.asarray(bq, np.float32)[hs].reshape(2, 128).T),
            "bk_p": np.ascontiguousarray(
                np.asarray(bk, np.float32)[hs].reshape(2, 128).T),
            "bv_b": np.ascontiguousarray(
                np.broadcast_to(np.asarray(bv, np.float32)[hs], (128, DPC))),
            "bo_b": np.ascontiguousarray(bo_b),
        })
    return in_maps


_NC = None


def build_nc(loop_n=None):
    nc = bacc.Bacc(
        "TRN2",
        target_bir_lowering=False,
        debug=False,
        enable_asserts=False,
        num_devices=N_CORES,
    )
    ins = {}
    for name in ("xq_t", "xk_t", "xv_t"):
        ins[name] = nc.dram_tensor(
            name, [D_MODEL, S], BF16, kind="ExternalInput").ap()
    for name in ("wq_t", "wk_t", "wv_t"):
        ins[name] = nc.dram_tensor(
            name, [D_MODEL, DPC], BF16, kind="ExternalInput").ap()
    ins["wo_t"] = nc.dram_tensor(
        "wo_t", [DPC, D_MODEL], BF16, kind="ExternalInput").ap()
    ins["bq_p"] = nc.dram_tensor("bq_p", [128, 2], F32, kind="ExternalInput").ap()
    ins["bk_p"] = nc.dram_tensor("bk_p", [128, 2], F32, kind="ExternalInput").ap()
    ins["bv_b"] = nc.dram_tensor("bv_b", [128, DPC], F32, kind="ExternalInput").ap()
    ins["bo_b"] = nc.dram_tensor(
        "bo_b", [128, D_MODEL], F32, kind="ExternalInput").ap()
    out_ap = nc.dram_tensor("out", [S, D_MODEL], F32, kind="ExternalOutput").ap()
    with tile.TileContext(nc) as tc:
        build_mha(tc, ins, out_ap, loop_n=loop_n)
    nc.compile()
    return nc


def _get_nc():
    global _NC
    if _NC is None:
        _NC = build_nc()
    return _NC


def run_sharded(inputs, trace=False):
    nc = _get_nc()
    in_maps = shard_inputs(**inputs)
    res = bass_utils.run_bass_kernel_spmd(
        nc, in_maps, core_ids=list(range(N_CORES)), trace=trace
    )
    acc = np.zeros((B, S, D_MODEL), np.float64)
    for c in range(N_CORES):
        acc[c // 4] += res.results[c]["out"].astype(np.float64)
    return acc.astype(np.float32), res


def kernel(**inputs):
    out, _ = run_sharded(inputs, trace=False)
    return out
